# revision 33
# baseline (speedup 1.0000x reference)
"""GIN (3-layer) Trainium2 Bass kernel, 8-core SPMD.  v8

Sharding: nodes (and incident edges, by dst) partitioned across 8 cores;
segment_sum local per dst shard; MLP weights replicated; features exchanged
between layers with a row-split AllGather (wave A = epilogue blocks 0-3,
wave B = 4-9), CC-A fired as soon as the first four epilogues land.

Key mechanics:
  - 128-dst (block) dedup of gather rows; chunk pairing crosses the A/B wave
    boundary (single even-ceil per block, ~162 chunks); per-block indirect
    dma_gather of fp8 rows (values/16) so each block's selector matmuls start
    as soon as its rows land; a dummy gather hoists the gpsimd ucode library
    load off the critical path.
  - segment-sum on the PE in fp8 DoubleRow mode against a host-built one-hot
    selector S (entries 16*mult, resident in SBUF).
  - node-major resident h (bf16): Z = agg(PSUM) + h in one DVE add; only Z is
    transposed (PE) for the feature-major first MLP GEMM.
  - second MLP GEMM per node-block with Y1^T as the stationary operand ->
    node-major output directly (no epilogue transposes); bias via a K=1
    matmul row; epilogue writes fp8 h/16 (scalar) for the next exchange.
"""

import os
import sys
from contextlib import ExitStack

import numpy as np

for _p in ("/opt/trn_rl_repo", "/root/.axon_site/_ro/trn_rl_repo"):
    if os.path.isdir(_p) and _p not in sys.path:
        sys.path.append(_p)

import ml_dtypes

N_NODES = 10000
N_EDGES = 160000
D = 512
N_LAYERS = 3
CORES = 8
SHARD = N_NODES // CORES          # 1250 nodes per core
PADS = 1280                       # padded shard (multiple of 128)
NB = PADS // 128                  # dst blocks per core (10)
NBA, NBB = 4, 6                   # wave A blocks (0-3), wave B blocks (4-9)
ASPL = NBA * 128                  # A-wave rows per shard
BSPL = NBB * 128                  # B-wave rows per shard
FSCALE = 16.0                     # fp8 feature scale (folded into S)

BF16 = ml_dtypes.bfloat16
F8 = ml_dtypes.float8_e4m3fn

LAST_RESULTS = None


def _prep_host(x, edge_index, Ws, bs):
    x = np.asarray(x, np.float32)
    src = np.asarray(edge_index[0], np.int64)
    dst = np.asarray(edge_index[1], np.int64)
    Ws = np.asarray(Ws, np.float32)
    bs = np.asarray(bs, np.float32)

    g = (src // SHARD) * PADS + (src % SHARD)   # padded global src row
    owner = dst // SHARD
    dloc = dst - owner * SHARD
    blk = dloc // 128
    j = dloc % 128

    # Uniform per-block chunk counts (max over cores).
    CA = np.zeros(NB, np.int64)
    CB = np.zeros(NB, np.int64)
    for c in range(CORES):
        for b in range(NB):
            m = (owner == c) & (blk == b)
            u = np.unique(g[m])
            nA = int((u % PADS < ASPL).sum())
            nB = len(u) - nA
            CA[b] = max(CA[b], -(-nA // 128))
            CB[b] = max(CB[b], -(-nB // 128))
    C_A = [int(v) for v in CA]
    C_B = [int(v) for v in CB]
    C_T = [C_A[b] + C_B[b] for b in range(NB)]
    OFF = np.concatenate([[0], np.cumsum(C_T)]).astype(np.int64)
    TOTC = int(OFF[-1])
    CBMAX = max(C_T)

    # fp8 wave layout of x (x/16), only for host pre-gather of layer 0.
    xa = np.zeros((CORES * ASPL, D), F8)
    xb = np.zeros((CORES * BSPL, D), F8)
    for o in range(CORES):
        xs = (x[o * SHARD:(o + 1) * SHARD] / FSCALE).astype(F8)
        xa[o * ASPL:o * ASPL + ASPL] = xs[:ASPL]
        xb[o * BSPL:o * BSPL + SHARD - ASPL] = xs[ASPL:]

    Wd = np.ascontiguousarray(Ws.reshape(2 * N_LAYERS, D, D).astype(BF16))
    bT = np.ascontiguousarray(
        bs[:, 0].reshape(N_LAYERS, 4, 128).transpose(2, 0, 1).reshape(128, 4 * N_LAYERS))
    b1r = np.ascontiguousarray(bs[:, 1].reshape(1, N_LAYERS * D).astype(BF16))
    ones1 = np.ones((1, 128), BF16)
    identb = np.eye(128, dtype=BF16)

    in_maps = []
    for c in range(CORES):
        Scnt = np.zeros((128, TOTC, 128), np.int16)
        idxd = np.zeros((128, TOTC * 8), np.int16)
        xgc = np.zeros((128, TOTC * D), F8)
        for b in range(NB):
            m = (owner == c) & (blk == b)
            eg = g[m]
            uniq, inv = np.unique(eg, return_inverse=True)
            ub = uniq % PADS >= ASPL
            nA = int((~ub).sum())
            nB = len(uniq) - nA
            cA = C_A[b]
            posmap = np.empty(len(uniq), np.int64)
            posmap[~ub] = np.arange(nA)
            posmap[ub] = cA * 128 + np.arange(nB)
            pos = posmap[inv]
            np.add.at(Scnt, (pos % 128, OFF[b] + pos // 128, j[m]), 1)
            # owner-major wave-buffer row ids (AllGather output layout)
            uo = (uniq // PADS).astype(np.int64)
            ur = (uniq % PADS).astype(np.int64)
            rb = ur - ASPL
            rows_a = uo * ASPL + ur
            rows_b = uo * BSPL + rb
            u_rows = np.where(ub, rows_b, rows_a).astype(np.int16)
            glist = np.zeros(C_T[b] * 128, np.int16)
            glist[:nA] = u_rows[~ub]
            glist[cA * 128:cA * 128 + nB] = u_rows[ub]
            w = glist.reshape(C_T[b] * 8, 16).T
            idxd[:, OFF[b] * 8:(OFF[b] + C_T[b]) * 8] = np.tile(w, (8, 1))
            # layer-0 host pre-gather (owner-order sources)
            gwa = uo * ASPL + ur
            gwb = uo * BSPL + rb
            rows = np.empty((C_T[b] * 128, D), F8)
            rows[:] = xa[0]
            rows[:nA] = xa[gwa[~ub]]
            rows[cA * 128:cA * 128 + nB] = xb[gwb[ub]]
            xgc[:, OFF[b] * D:(OFF[b] + C_T[b]) * D] = (
                rows.reshape(C_T[b], 128, D).transpose(1, 0, 2).reshape(128, C_T[b] * D))
        Sd = (Scnt.astype(np.float32) * FSCALE).astype(F8)
        xn = np.zeros((NB, 128, D), np.float32)
        xn.reshape(-1, D)[:SHARD] = x[c * SHARD:(c + 1) * SHARD]
        hb = np.ascontiguousarray(
            xn.transpose(1, 0, 2).reshape(128, NB * D).astype(BF16))
        in_maps.append({
            "xgc": xgc,
            "hb16": hb,
            "Wd": Wd,
            "bT": bT,
            "b1r": b1r,
            "ones1": ones1,
            "identb": identb,
            "Sd": Sd,
            "idxd": idxd,
        })
    return in_maps, C_A, C_B, C_T, [int(v) for v in OFF], CBMAX


def build_program(C_A, C_B, C_T, OFF, CBMAX):
    import concourse.bacc as bacc
    import concourse.bass as bass
    import concourse.mybir as mybir
    import concourse.tile as tile

    dt = mybir.dt
    f32, bf16, f8, i16 = dt.float32, dt.bfloat16, dt.float8e4, dt.int16
    AF = mybir.ActivationFunctionType
    DR = mybir.MatmulPerfMode.DoubleRow
    TOTC = OFF[-1]
    NEX = N_LAYERS - 1            # exchanges (after layers 0, 1)
    # per-exchange arrival counts: 7 senders x blocks x 2 sem incs
    RSA_INC = 7 * NBA * 2
    RSB_INC = 7 * NBB * 2

    nc = bacc.Bacc("TRN2", target_bir_lowering=False, debug=False,
                   enable_asserts=False, num_devices=CORES, num_swdge_queues=4)

    xgc = nc.dram_tensor("xgc", [128, TOTC * D], f8, kind="ExternalInput")
    hb16d = nc.dram_tensor("hb16", [128, NB * D], bf16, kind="ExternalInput")
    Wd = nc.dram_tensor("Wd", [2 * N_LAYERS, D, D], bf16, kind="ExternalInput")
    bTd = nc.dram_tensor("bT", [128, 4 * N_LAYERS], f32, kind="ExternalInput")
    b1rd = nc.dram_tensor("b1r", [1, N_LAYERS * D], bf16, kind="ExternalInput")
    ones1d = nc.dram_tensor("ones1", [1, 128], bf16, kind="ExternalInput")
    identbd = nc.dram_tensor("identb", [128, 128], bf16, kind="ExternalInput")
    Sdr = nc.dram_tensor("Sd", [128, TOTC, 128], f8, kind="ExternalInput")
    idxd = nc.dram_tensor("idxd", [128, TOTC * 8], i16, kind="ExternalInput")
    outd = nc.dram_tensor("out", [PADS, D], f32, kind="ExternalOutput")

    NCHUNK = [(0, 512), (512, 512), (1024, PADS - 1024)]

    rsem = nc.alloc_semaphore("rsem")     # remote-arrival counts (not waited on)
    lsem = nc.alloc_semaphore("lsem")     # local send completions (not waited on)
    f_sem = nc.alloc_semaphore("f_sem")   # fence-copy completions (waited on)

    with tile.TileContext(nc) as tc, ExitStack() as ctx:
        p_const = ctx.enter_context(tc.tile_pool(name="const", bufs=1))
        p_big = ctx.enter_context(tc.tile_pool(name="big", bufs=1))
        p_g = ctx.enter_context(tc.tile_pool(name="gth", bufs=10))
        p_z = ctx.enter_context(tc.tile_pool(name="z", bufs=14))
        p_hbf = ctx.enter_context(tc.tile_pool(name="hbf", bufs=2))
        p_ot = ctx.enter_context(tc.tile_pool(name="ot", bufs=2))
        p_aggps = ctx.enter_context(tc.tile_pool(name="aggps", bufs=2, space="PSUM"))
        p_tps = ctx.enter_context(tc.tile_pool(name="tps", bufs=2, space="PSUM"))
        p_mlpps = ctx.enter_context(tc.tile_pool(name="mlpps", bufs=3, space="PSUM"))
        p_dram = ctx.enter_context(tc.tile_pool(name="dram", bufs=1, space="DRAM"))

        # ---- constants (first-needed-first per queue) ----------------------
        identb = p_const.tile([128, 128], bf16)
        S = p_big.tile([128, TOTC, 128], f8)
        hb16 = p_big.tile([128, NB, D], bf16)
        for b in range(2):
            nc.scalar.dma_start(S[:, OFF[b]:OFF[b + 1], :],
                                Sdr.ap()[:, OFF[b]:OFF[b + 1], :])

        bt = p_const.tile([128, 4 * N_LAYERS], f32)
        nc.scalar.dma_start(bt[:], bTd.ap())
        b1r = p_const.tile([1, N_LAYERS * D], bf16)
        nc.scalar.dma_start(b1r[:], b1rd.ap())
        ones1 = p_const.tile([1, 128], bf16)
        nc.scalar.dma_start(ones1[:], ones1d.ap())

        for b in range(2, NB):
            nc.scalar.dma_start(S[:, OFF[b]:OFF[b + 1], :],
                                Sdr.ap()[:, OFF[b]:OFF[b + 1], :])
        idxs = p_const.tile([128, TOTC * 8], i16)
        nc.scalar.dma_start(idxs[:], idxd.ap())

        Wts = {}
        for l in range(N_LAYERS):
            Wts[l] = (p_big.tile([128, 4, D], bf16, name=f"W0t{l}"),
                      p_big.tile([128, 4, D], bf16, name=f"W1t{l}"))

        def emit_wload(l):
            W0t, W1t = Wts[l]
            for kc in range(4):
                nc.sync.dma_start(W0t[:, kc, :], Wd.ap()[2 * l, kc * 128:(kc + 1) * 128, :])
                nc.sync.dma_start(W1t[:, kc, :], Wd.ap()[2 * l + 1, kc * 128:(kc + 1) * 128, :])

        ZT = p_big.tile([128, 4, PADS], bf16)
        Y1T = p_big.tile([128, 4, PADS], bf16)

        # warmup collective (absorbs first-CC init latency)
        wa_in = p_dram.tile([128, 16], bf16, name="wa_in")
        wa_out = p_dram.tile([128 * CORES, 16], bf16, addr_space="Shared", name="wa_out")
        nc.sync.dma_start(wa_in[:, :], identbd.ap()[:, 0:16])
        nc.gpsimd.collective_compute(
            "AllGather", mybir.AluOpType.bypass,
            replica_groups=[list(range(CORES))],
            ins=[wa_in.opt()], outs=[wa_out.opt()])

        # exchange staging + AllGather outputs (owner-major rows)
        hsh_a = [p_dram.tile([ASPL, D], f8, name=f"hsa{l}") for l in range(NEX)]
        hsh_b = [p_dram.tile([BSPL, D], f8, name=f"hsb{l}") for l in range(NEX)]
        ag_a = [p_dram.tile([CORES * ASPL, D], f8, addr_space="Shared",
                            name=f"aga{l}") for l in range(NEX)]
        ag_b = [p_dram.tile([CORES * BSPL, D], f8, addr_space="Shared",
                            name=f"agb{l}") for l in range(NEX)]

        def emit_cc(ins, outs):
            nc.gpsimd.collective_compute(
                "AllGather", mybir.AluOpType.bypass,
                replica_groups=[list(range(CORES))],
                ins=[ins.opt()], outs=[outs.opt()])

        qctr = [0]
        gtiles = {}

        def emit_gather(l, b, half):
            cA = C_A[b]
            cH = cA if half == 0 else C_T[b] - cA
            o = 0 if half == 0 else cA
            if half == 0:
                gt = p_g.tile([128, CBMAX, D], f8, tag="g", name="g")
                gtiles[(l, b)] = gt
            else:
                gt = gtiles[(l, b)]
            src = (ag_a[l - 1] if half == 0 else ag_b[l - 1])[:, :]
            qn = qctr[0] % 3
            qctr[0] += 1
            nc.gpsimd.dma_gather(
                out_ap=gt[:, o:o + cH, :],
                in_ap=src,
                idxs_ap=idxs[:, (OFF[b] + o) * 8:(OFF[b] + o + cH) * 8],
                num_idxs=cH * 128,
                num_idxs_reg=cH * 128,
                elem_size=D,
                single_packet=False,
                queue_num=qn,
            )

        def emit_load0(b):
            gt = p_g.tile([128, CBMAX, D], f8, tag="g", name="g")
            gtiles[(0, b)] = gt
            nc.sync.dma_start(gt[:, :C_T[b], :],
                              xgc.ap()[:, OFF[b] * D:(OFF[b] + C_T[b]) * D])

        zparts = {}

        def _agg_mms(ps, b, o, n, gt):
            # DR pairs + a lone normal-mode matmul for an odd tail; the
            # segment is a complete PSUM accumulation group (start..stop).
            npair = n // 2
            for p in range(npair):
                nc.tensor.matmul(ps[:], lhsT=S[:, OFF[b] + o + 2 * p:OFF[b] + o + 2 * p + 2, :],
                                 rhs=gt[:, o + 2 * p:o + 2 * p + 2, :],
                                 start=(p == 0), stop=(not (n & 1)) and p == npair - 1,
                                 perf_mode=DR)
            if n & 1:
                nc.tensor.matmul(ps[:], lhsT=S[:, OFF[b] + o + n - 1, :],
                                 rhs=gt[:, o + n - 1, :],
                                 start=(n == 1), stop=True)

        def emit_aggA(l, b):
            # A-wave partial aggregate + self term -> bf16 zb (fills the
            # boundary hole while wave B is still in CC flight)
            gt = gtiles[(l, b)]
            ps = p_aggps.tile([128, D], f32, tag="agg", name="ps")
            _agg_mms(ps, b, 0, C_A[b], gt)
            zb = p_z.tile([128, D], bf16, tag="z", name="zb")
            nc.vector.tensor_add(zb[:], ps[:], hb16[:, b, :])
            zparts[(l, b)] = zb

        def emit_aggB(l, b):
            gt = gtiles.pop((l, b))
            zb = zparts.pop((l, b))
            ps = p_aggps.tile([128, D], f32, tag="agg", name="ps")
            _agg_mms(ps, b, C_A[b], C_B[b], gt)
            zb2 = p_z.tile([128, D], bf16, tag="z", name="zb2")
            nc.vector.tensor_add(zb2[:], ps[:], zb[:])
            for fc in range(4):
                pt = p_tps.tile([128, 128], bf16, tag="t", name="pt")
                nc.tensor.transpose(pt[:], zb2[:, fc * 128:(fc + 1) * 128], identb[:])
                nc.vector.tensor_copy(ZT[:, fc, b * 128:(b + 1) * 128], pt[:])

        def emit_j0(l, c):
            nofs, nw = NCHUNK[c]
            W0t = Wts[l][0]
            for mc in range(4):
                ps2 = p_mlpps.tile([128, D], f32, tag="mlp", name="ps2")
                for kc in range(4):
                    nc.tensor.matmul(
                        ps2[:, :nw],
                        lhsT=W0t[:, kc, mc * 128:(mc + 1) * 128],
                        rhs=ZT[:, kc, nofs:nofs + nw],
                        start=(kc == 0), stop=(kc == 3))
                col = l * 4 + mc
                nc.scalar.activation(Y1T[:, mc, nofs:nofs + nw],
                                     ps2[:, :nw], AF.Relu, bias=bt[:, col:col + 1])

        def emit_j1(l, b):
            W1t = Wts[l][1]
            ps2 = p_mlpps.tile([128, D], f32, tag="mlp", name="ps2")
            nc.tensor.matmul(ps2[:], lhsT=ones1[:1, :],
                             rhs=b1r[:1, l * D:(l + 1) * D], start=True, stop=False)
            for kc in range(4):
                nc.tensor.matmul(ps2[:],
                                 lhsT=Y1T[:, kc, b * 128:(b + 1) * 128],
                                 rhs=W1t[:, kc, :],
                                 start=False, stop=(kc == 3))
            if l < N_LAYERS - 1:
                hf = p_hbf.tile([128, D], f8, tag="hbf", name="hf")
                nc.scalar.activation(hf[:], ps2[:], AF.Relu, scale=1.0 / FSCALE)
                nc.scalar.activation(hb16[:, b, :], ps2[:], AF.Relu)
                if b < NBA:
                    nc.sync.dma_start(hsh_a[l][b * 128:(b + 1) * 128, :], hf[:])
                else:
                    nc.sync.dma_start(hsh_b[l][(b - NBA) * 128:(b - NBA + 1) * 128, :], hf[:])
            else:
                ot = p_ot.tile([128, D], f32, tag="ot", name="ot")
                nc.scalar.activation(ot[:], ps2[:], AF.Identity)
                nc.sync.dma_start(outd.ap()[b * 128:(b + 1) * 128, :], ot[:])

        # ---- layer 0 loads -------------------------------------------------
        emit_load0(0)
        nc.sync.dma_start(identb[:], identbd.ap())
        emit_load0(1)
        nc.sync.dma_start(hb16[:, :, :], hb16d.ap())
        for b in range(2, 4):
            emit_load0(b)
        emit_wload(0)
        for b in range(4, NB):
            emit_load0(b)
        emit_wload(1)
        emit_wload(2)

        # ---- layers --------------------------------------------------------
        for b in range(NB):
            emit_aggA(0, b)
        for l in range(N_LAYERS):
            for b in range(4):
                emit_aggB(l, b)
            emit_j0(l, 0)
            # j1 + epilogue of wave-A blocks as early as possible -> CC-A
            for b in range(4):
                emit_j1(l, b)
            if l < N_LAYERS - 1:
                emit_cc(hsh_a[l], ag_a[l])
            for b in range(4, 8):
                emit_aggB(l, b)
            emit_j0(l, 1)
            for b in range(4, 8):
                emit_j1(l, b)
            emit_aggB(l, 8)
            emit_aggB(l, 9)
            emit_j0(l, 2)
            emit_j1(l, 8)
            emit_j1(l, 9)
            if l < N_LAYERS - 1:
                emit_cc(hsh_b[l], ag_b[l])

                # dummy gather: hoist the mlp-ucode library switch off the
                # critical path (runs during the CC flight)
                gdummy = p_z.tile([128, 1, D], f8, tag="gd", name="gd")
                nc.gpsimd.dma_gather(
                    out_ap=gdummy[:, :, :], in_ap=xgc.ap().rearrange("p (x d) -> (p x) d", d=D),
                    idxs_ap=idxs[:, 0:8], num_idxs=128, num_idxs_reg=128,
                    elem_size=D, single_packet=False, queue_num=0)

                # next layer's gathers (A-calls lead; block-level deps)
                order = []
                na, nb_ = 0, 0
                for _ in range(2 * NB):
                    if na < NB and (na < 3 or na - nb_ < 4 or nb_ >= NB):
                        order.append((na, 0)); na += 1
                    else:
                        order.append((nb_, 1)); nb_ += 1
                for b, half in order:
                    emit_gather(l + 1, b, half)
                # A-wave partial aggs of the next layer fill the CC-B hole
                for b in range(NB):
                    emit_aggA(l + 1, b)

    nc.compile()
    return nc


def kernel(**inputs):
    global LAST_RESULTS
    from concourse import bass_utils

    in_maps, C_A, C_B, C_T, OFF, CBMAX = _prep_host(
        inputs["x"], inputs["edge_index"], inputs["Ws"], inputs["bs"])
    nc = build_program(C_A, C_B, C_T, OFF, CBMAX)
    res = bass_utils.run_bass_kernel_spmd(
        nc, in_maps, core_ids=list(range(CORES)),
        trace=bool(int(os.environ.get("GIN_TRACE", "0"))),
        tmpdir=os.environ.get("GIN_TMPDIR"),
    )
    LAST_RESULTS = res
    out = np.empty((N_NODES, D), np.float32)
    for c in range(CORES):
        out[c * SHARD:(c + 1) * SHARD] = res.results[c]["out"][:SHARD]
    return out
